# revision 11
# baseline (speedup 1.0000x reference)
"""BilinearInteraction Trainium2 kernel (8 NeuronCores, batch-sharded).

out[b, p=(i,j), d] = x[b, i, d] * (x @ W)[b, j, d]  for the 496 upper-tri
pairs of F=32 fields; x [4096, 32, 64] f32, W [64, 64] f32.

Per core: 512 batch rows, processed as 4 tiles of 128 (batch on SBUF
partitions). Per tile:
  - vid = x @ W via 16 PE pair-block transposes ([128,128] f-pair blocks
    -> PSUM) + 16 matmuls against a block-diag [[W,0],[0,W]] so two
    fields resolve per instruction.
  - pairwise Hadamard on DVE: one tensor_mul per field i covering all
    j>i at once, broadcasting x[:,i,:] over the j axis with a stride-0
    access pattern (fp32 TT runs 1x, so long free dims amortize the
    per-instruction overhead).
  - output staged in SBUF in 4 chunks of 124 pairs, each DMA'd as one
    ~4 MB contiguous-per-partition transfer (31 KB/partition lines).
"""

import sys

if "/opt/trn_rl_repo" not in sys.path:
    sys.path.insert(0, "/opt/trn_rl_repo")

import numpy as np

import concourse.bass as bass
import concourse.mybir as mybir
import concourse.tile as tile
from concourse import bacc
from concourse.bass_utils import run_bass_kernel_spmd

B, F, D = 4096, 32, 64
P = F * (F - 1) // 2  # 496
NCORES = 8
BSH = B // NCORES  # 512 batch rows per core
BT = 128  # batch tile (SBUF partitions)
NTILES = BSH // BT  # 4

f32 = mybir.dt.float32

# pair-block offsets: block i = pairs (i, j) for j in i+1..F-1
POFF = [0]
for i in range(F - 1):
    POFF.append(POFF[-1] + (F - 1 - i))
CHUNKS = [(0, 124), (124, 248), (248, 372), (372, 496)]


def _emit(tc, nc, x_d, w2_d, i128_d, out_d):
    with (
        tc.tile_pool(name="const", bufs=1) as const_pool,
        tc.tile_pool(name="xp", bufs=4) as x_pool,
        tc.tile_pool(name="vidp", bufs=2) as vid_pool,
        tc.tile_pool(name="xtp", bufs=4) as xt_pool,
        tc.tile_pool(name="outp", bufs=3) as out_pool,
        tc.tile_pool(name="ps_t", bufs=2, space="PSUM") as ps_t,
        tc.tile_pool(name="ps_m", bufs=2, space="PSUM") as ps_m,
    ):
        # inputs ride the scalar-engine HWDGE ring; outputs own the sync
        # HWDGE ring (a shared FIFO would park tile t+1's x load behind
        # tile t's ~40 us of output stores and starve the DVE).
        ident = const_pool.tile([128, 128], f32)
        nc.scalar.dma_start(out=ident[:], in_=i128_d[:])
        w2 = const_pool.tile([128, 128], f32)
        nc.scalar.dma_start(out=w2[:], in_=w2_d[:])
        # each x tile loads in two halves, high fields first: the
        # first-processed chunk (pairs 372..496, blocks i>=15) only reads
        # x fields >=16 and vid f-pairs >=8, so compute starts after half
        # a tile load.
        x_ts = []
        for t in range(NTILES):
            x_t = x_pool.tile([128, F * D], f32, tag="xt")
            for flo, fhi in ((F // 2, F), (0, F // 2)):
                nc.scalar.dma_start(
                    out=x_t[:, flo * D : fhi * D].rearrange(
                        "p (f d) -> p f d", d=D
                    ),
                    in_=x_d[t * BT : (t + 1) * BT, flo:fhi, :],
                )
            x_ts.append(x_t)

        for t in range(NTILES):
            b0 = t * BT
            x_t = x_ts[t]
            x3 = x_t[:].rearrange("p (f d) -> p f d", d=D)

            # vid f-pairs in descending order: the first-processed chunk
            # (blocks i=19..30) only needs fp>=10, so DVE + output DMA
            # start long before the whole vid tile is done.
            vid_t = vid_pool.tile([128, F * D], f32, tag="vidt")
            for fp in reversed(range(F // 2)):
                xT_ps = ps_t.tile([128, 128], f32, tag="xtps")
                nc.tensor.transpose(
                    xT_ps[:], x_t[:, fp * 128 : (fp + 1) * 128], ident[:]
                )
                xT_sb = xt_pool.tile([128, 128], f32, tag="xtsb")
                nc.scalar.copy(xT_sb[:], xT_ps[:])
                vid_ps = ps_m.tile([128, 128], f32, tag="vidps")
                nc.tensor.matmul(vid_ps[:], xT_sb[:], w2[:], start=True, stop=True)
                nc.scalar.copy(vid_t[:, fp * 128 : (fp + 1) * 128], vid_ps[:])
            vid3 = vid_t[:].rearrange("p (f d) -> p f d", d=D)

            for ci, (c0, c1) in enumerate(reversed(CHUNKS)):
                npair = c1 - c0
                o_t = out_pool.tile([128, npair * D], f32, tag="outs")
                o3 = o_t[:].rearrange("p (q d) -> p q d", d=D)
                for i in reversed(range(F - 1)):
                    blk0, blk1 = POFF[i], POFF[i + 1]
                    lo, hi = max(blk0, c0), min(blk1, c1)
                    if lo >= hi:
                        continue
                    nj = hi - lo
                    j0 = i + 1 + (lo - blk0)
                    nc.vector.tensor_mul(
                        o3[:, lo - c0 : hi - c0, :],
                        x3[:, i : i + 1, :].broadcast_to((128, nj, D)),
                        vid3[:, j0 : j0 + nj, :],
                    )
                nc.sync.dma_start(out=out_d[b0 : b0 + BT, c0:c1, :], in_=o3[:])


def build_nc():
    nc = bacc.Bacc("TRN2", target_bir_lowering=False, debug=False)
    x_d = nc.dram_tensor("x", [BSH, F, D], f32, kind="ExternalInput")
    w2_d = nc.dram_tensor("W2", [128, 128], f32, kind="ExternalInput")
    i128_d = nc.dram_tensor("I128", [128, 128], f32, kind="ExternalInput")
    out_d = nc.dram_tensor("out", [BSH, P, D], f32, kind="ExternalOutput")
    with tile.TileContext(nc) as tc:
        _emit(tc, nc, x_d.ap(), w2_d.ap(), i128_d.ap(), out_d.ap())
    nc.compile()
    return nc


_NC = None


def kernel(x: np.ndarray, W: np.ndarray, _trace=False, _trace_kwargs=None):
    global _NC
    if _NC is None:
        _NC = build_nc()
    x = np.ascontiguousarray(x, dtype=np.float32)
    W = np.ascontiguousarray(W, dtype=np.float32)
    w2 = np.zeros((128, 128), dtype=np.float32)
    w2[:64, :64] = W
    w2[64:, 64:] = W
    i128 = np.eye(128, dtype=np.float32)
    in_maps = [
        {"x": x[i * BSH : (i + 1) * BSH], "W2": w2, "I128": i128}
        for i in range(NCORES)
    ]
    res = run_bass_kernel_spmd(
        _NC,
        in_maps,
        core_ids=list(range(NCORES)),
        trace=_trace,
        **(_trace_kwargs or {}),
    )
    out = np.concatenate([res.results[i]["out"] for i in range(NCORES)], axis=0)
    if _trace:
        return out, res
    return out


# revision 12
# speedup vs baseline: 1.1866x; 1.1866x over previous
"""BilinearInteraction Trainium2 kernel (8 NeuronCores, batch-sharded).

out[b, p=(i,j), d] = x[b, i, d] * (x @ W)[b, j, d]  for the 496 upper-tri
pairs of F=32 fields; x [4096, 32, 64] f32, W [64, 64] f32.

Per core: 512 batch rows, processed as 4 tiles of 128 (batch on SBUF
partitions). Per tile:
  - vid = x @ W via 16 PE pair-block transposes ([128,128] f-pair blocks
    -> PSUM) + 16 matmuls against a block-diag [[W,0],[0,W]] so two
    fields resolve per instruction.
  - pairwise Hadamard on DVE: one tensor_mul per field i covering all
    j>i at once, broadcasting x[:,i,:] over the j axis with a stride-0
    access pattern (fp32 TT runs 1x, so long free dims amortize the
    per-instruction overhead).
  - output staged in SBUF in 4 chunks of 124 pairs, each DMA'd as one
    ~4 MB contiguous-per-partition transfer (31 KB/partition lines).
"""

import sys

if "/opt/trn_rl_repo" not in sys.path:
    sys.path.insert(0, "/opt/trn_rl_repo")

import numpy as np

import concourse.bass as bass
import concourse.mybir as mybir
import concourse.tile as tile
from concourse import bacc
from concourse.bass_utils import run_bass_kernel_spmd

B, F, D = 4096, 32, 64
P = F * (F - 1) // 2  # 496
NCORES = 8
BSH = B // NCORES  # 512 batch rows per core
BT = 128  # batch tile (SBUF partitions)
NTILES = BSH // BT  # 4

f32 = mybir.dt.float32

# pair-block offsets: block i = pairs (i, j) for j in i+1..F-1
POFF = [0]
for i in range(F - 1):
    POFF.append(POFF[-1] + (F - 1 - i))
CHUNKS = [(0, 124), (124, 248), (248, 372), (372, 496)]


def _emit(tc, nc, x_d, w2_d, i128_d, out_d):
    with (
        tc.tile_pool(name="const", bufs=1) as const_pool,
        tc.tile_pool(name="xp", bufs=4) as x_pool,
        tc.tile_pool(name="vidp", bufs=2) as vid_pool,
        tc.tile_pool(name="xtp", bufs=4) as xt_pool,
        tc.tile_pool(name="outp", bufs=3) as out_pool,
        tc.tile_pool(name="ps_t", bufs=2, space="PSUM") as ps_t,
        tc.tile_pool(name="ps_m", bufs=2, space="PSUM") as ps_m,
    ):
        # inputs ride the scalar-engine HWDGE ring; outputs own the sync
        # HWDGE ring (a shared FIFO would park tile t+1's x load behind
        # tile t's ~40 us of output stores and starve the DVE).
        ident = const_pool.tile([128, 128], f32)
        nc.scalar.dma_start(out=ident[:], in_=i128_d[:])
        w2 = const_pool.tile([128, 128], f32)
        nc.scalar.dma_start(out=w2[:], in_=w2_d[:])
        x_ts = []
        for t in range(NTILES):
            x_t = x_pool.tile([128, F * D], f32, tag="xt")
            nc.scalar.dma_start(
                out=x_t[:].rearrange("p (f d) -> p f d", d=D),
                in_=x_d[t * BT : (t + 1) * BT, :, :],
            )
            x_ts.append(x_t)

        for t in range(NTILES):
            b0 = t * BT
            x_t = x_ts[t]
            x3 = x_t[:].rearrange("p (f d) -> p f d", d=D)

            # vid f-pairs in descending order: the first-processed chunk
            # (blocks i=19..30) only needs fp>=10, so DVE + output DMA
            # start long before the whole vid tile is done.
            vid_t = vid_pool.tile([128, F * D], f32, tag="vidt")
            for fp in reversed(range(F // 2)):
                xT_ps = ps_t.tile([128, 128], f32, tag="xtps")
                nc.tensor.transpose(
                    xT_ps[:], x_t[:, fp * 128 : (fp + 1) * 128], ident[:]
                )
                xT_sb = xt_pool.tile([128, 128], f32, tag="xtsb")
                nc.scalar.copy(xT_sb[:], xT_ps[:])
                vid_ps = ps_m.tile([128, 128], f32, tag="vidps")
                nc.tensor.matmul(vid_ps[:], xT_sb[:], w2[:], start=True, stop=True)
                nc.scalar.copy(vid_t[:, fp * 128 : (fp + 1) * 128], vid_ps[:])
            vid3 = vid_t[:].rearrange("p (f d) -> p f d", d=D)

            for ci, (c0, c1) in enumerate(reversed(CHUNKS)):
                npair = c1 - c0
                o_t = out_pool.tile([128, npair * D], f32, tag="outs")
                o3 = o_t[:].rearrange("p (q d) -> p q d", d=D)
                for i in reversed(range(F - 1)):
                    blk0, blk1 = POFF[i], POFF[i + 1]
                    lo, hi = max(blk0, c0), min(blk1, c1)
                    if lo >= hi:
                        continue
                    nj = hi - lo
                    j0 = i + 1 + (lo - blk0)
                    nc.vector.tensor_mul(
                        o3[:, lo - c0 : hi - c0, :],
                        x3[:, i : i + 1, :].broadcast_to((128, nj, D)),
                        vid3[:, j0 : j0 + nj, :],
                    )
                nc.sync.dma_start(out=out_d[b0 : b0 + BT, c0:c1, :], in_=o3[:])


def build_nc():
    nc = bacc.Bacc("TRN2", target_bir_lowering=False, debug=False)
    x_d = nc.dram_tensor("x", [BSH, F, D], f32, kind="ExternalInput")
    w2_d = nc.dram_tensor("W2", [128, 128], f32, kind="ExternalInput")
    i128_d = nc.dram_tensor("I128", [128, 128], f32, kind="ExternalInput")
    out_d = nc.dram_tensor("out", [BSH, P, D], f32, kind="ExternalOutput")
    with tile.TileContext(nc) as tc:
        _emit(tc, nc, x_d.ap(), w2_d.ap(), i128_d.ap(), out_d.ap())
    nc.compile()
    return nc


_NC = None


def kernel(x: np.ndarray, W: np.ndarray, _trace=False, _trace_kwargs=None):
    global _NC
    if _NC is None:
        _NC = build_nc()
    x = np.ascontiguousarray(x, dtype=np.float32)
    W = np.ascontiguousarray(W, dtype=np.float32)
    w2 = np.zeros((128, 128), dtype=np.float32)
    w2[:64, :64] = W
    w2[64:, 64:] = W
    i128 = np.eye(128, dtype=np.float32)
    in_maps = [
        {"x": x[i * BSH : (i + 1) * BSH], "W2": w2, "I128": i128}
        for i in range(NCORES)
    ]
    res = run_bass_kernel_spmd(
        _NC,
        in_maps,
        core_ids=list(range(NCORES)),
        trace=_trace,
        **(_trace_kwargs or {}),
    )
    out = np.concatenate([res.results[i]["out"] for i in range(NCORES)], axis=0)
    if _trace:
        return out, res
    return out
